# revision 1
# baseline (speedup 1.0000x reference)
"""Trainium2 Bass kernel for nn_DestSelectionPolicy (GNN edge softmax).

Math: att[e,c] = relu(x[row_e]@W[c,:64] + x[col_e]@W[c,64:] + b[c]);
segment-softmax over edges grouped by row (destination), per channel;
mask amount==0 edges; sum the 2 channels -> out[e].

Identity used: exp(s)/sum(exp(s)) == exp(s-m)/sum(exp(s-m)); s=relu(.) is in
[0, ~5] so the unshifted exp is fp32-safe and matches the reference within
rounding (reference's +1e-16 in the denominator is relative 1e-16 since the
max term contributes exp(0)=1).

Sharding: edges partitioned by destination row range, 6250 rows/core x 8
cores, so each node's softmax segment is device-local (no collective).
Per core the device:
  1. builds a node table uv[n] = [u0+b0, u1+b1, v0, v1] from x via PE matmuls
     (u = x@W[:, :64].T, v = x@W[:, 64:].T) and writes it to DRAM,
  2. for each [128-node x Dt-slot] grid tile, fetches v-pairs for every edge
     slot with indirect row-gather DMAs (one 16B table row per destination
     partition per slot), then relu/exp on ACT, masked segment sum + divide
     on DVE, and writes the per-edge grid back.
Host packs edges into the grids (nodes sorted by degree so per-tile slot
counts Dt hug the real degrees) and scatters grid outputs to edge order.
"""
import sys

sys.path.insert(0, "/opt/trn_rl_repo")

import numpy as np
import concourse.bass as bass
import concourse.mybir as mybir
from concourse.bass_utils import run_bass_kernel_spmd
from concourse.tile import TileContext
from concourse.vector_clock import ScopedClock
import concourse.tile as tile_mod

N = 50000
E = 1600000
D = 64
NC = 8
RPC = N // NC          # rows per core (6250)
RP = 6272              # padded rows per core (49*128)
NT = RP // 128         # node tiles per core (49)
NROWS_TBL = 50176      # table rows (392*128; cols 50000+ are zero padding)
XT = NROWS_TBL // 128  # x tiles for table build (392)
F32 = mybir.dt.float32
I32 = mybir.dt.int32

# ---------------------------------------------------------------- tile patch
_MAXW = 1


def _patched_drain_and_barrier(self, tick_clock, wait_clock):
    # this walrus build rejects >1 sync wait per instruction; chain nops
    carrier = self.nc.sync.nop(nofuse=True, hint="drain_waits")
    wait_clock.add_sem_waits(
        carrier.ins, ScopedClock({None: tick_clock.global_clock})
    )
    si = carrier.ins.sync_info
    waits = list(si.on_wait) if si is not None else []
    if si is not None:
        si.on_wait = waits[:_MAXW]
    for i in range(_MAXW, len(waits), _MAXW):
        nop = self.nc.sync.nop(nofuse=True, hint="drain_waits")
        if nop.ins.sync_info is None:
            nop.ins.sync_info = mybir.SyncInfo(on_wait=[], on_update=[])
        nop.ins.sync_info.on_wait = waits[i : i + _MAXW]
    self.nc.sync.drain()
    self.nc.all_engine_barrier()
    assert self.sems is not None
    popped = self.nc._tile_sem_poison_stack.pop()
    assert popped is self._sem_poison
    self.nc.clear_and_free_semaphores(list(self.sems.allocated().values()))
    self.nc.all_engine_barrier()


tile_mod.TileContext._drain_and_barrier = _patched_drain_and_barrier


def _split_waits(nc, maxw: int = _MAXW):
    for fn in nc.m.functions:
        for bb in fn.blocks:
            new_insts = []
            for inst in bb.instructions:
                si = inst.sync_info
                if si is not None and si.on_wait and len(si.on_wait) > maxw:
                    waits = list(si.on_wait)
                    si.on_wait = waits[-maxw:]
                    for i in range(0, len(waits) - maxw, maxw):
                        new_insts.append(
                            mybir.InstNoOp(
                                name=nc.get_next_instruction_name(),
                                engine=inst.engine,
                                sync_info=mybir.SyncInfo(
                                    on_wait=waits[i : i + maxw], on_update=[]
                                ),
                                text_hint="wait_split",
                            )
                        )
                new_insts.append(inst)
            bb.instructions[:] = new_insts


# ---------------------------------------------------------------- device code
_CACHE = {}


def _build_nc(dts):
    W_slots = max(dts)
    nc = bass.Bass()
    x_t = nc.declare_dram_parameter("x_t", [D, NROWS_TBL], F32, isOutput=False)
    wcat = nc.declare_dram_parameter("wcat", [D, 4], F32, isOutput=False)
    btile = nc.declare_dram_parameter("btile", [128, 4], F32, isOutput=False)
    idx_g = nc.declare_dram_parameter("idx_g", [RP, W_slots], I32, isOutput=False)
    u_idx = nc.declare_dram_parameter("u_idx", [RP, 1], I32, isOutput=False)
    valid_g = nc.declare_dram_parameter("valid_g", [RP, W_slots], F32, isOutput=False)
    mask_g = nc.declare_dram_parameter("mask_g", [RP, W_slots], F32, isOutput=False)
    out_g = nc.declare_dram_parameter("out_g", [RP, W_slots], F32, isOutput=True)
    uv = nc.dram_tensor("uv_tbl", [NROWS_TBL, 4], F32)

    G = 8  # x tiles per staging group
    with TileContext(nc) as tc:
        with (
            tc.tile_pool(name="consts", bufs=1) as cpool,
            tc.tile_pool(name="xc", bufs=3) as xpool,
            tc.tile_pool(name="ps", bufs=4, space="PSUM") as pspool,
            tc.tile_pool(name="st", bufs=3) as stpool,
            tc.tile_pool(name="edge", bufs=3) as epool,
            tc.tile_pool(name="vals", bufs=3) as vpool,
            tc.tile_pool(name="small", bufs=4) as spool,
        ):
            wc = cpool.tile([D, 4], F32, tag="wc")
            nc.sync.dma_start(out=wc[:], in_=wcat[:])
            bt = cpool.tile([128, 4], F32, tag="bt")
            nc.sync.dma_start(out=bt[:], in_=btile[:])

            # phase 1: uv table = [u0+b0, u1+b1, v0, v1] per node
            for g0 in range(0, XT, G):
                gn = min(G, XT - g0)
                xc = xpool.tile([D, 128 * gn], F32, tag="xc")
                nc.sync.dma_start(
                    out=xc[:], in_=x_t[:, g0 * 128 : (g0 + gn) * 128]
                )
                st = stpool.tile([128, 4 * gn], F32, tag="st")
                for g in range(gn):
                    ps = pspool.tile([128, 4], F32, tag="ps")
                    nc.tensor.matmul(
                        out=ps[:],
                        lhsT=xc[:, g * 128 : (g + 1) * 128],
                        rhs=wc[:],
                        start=True,
                        stop=True,
                    )
                    nc.vector.tensor_add(
                        out=st[:, g * 4 : (g + 1) * 4], in0=ps[:], in1=bt[:]
                    )
                nc.sync.dma_start(
                    out=uv[g0 * 128 : (g0 + gn) * 128, :].rearrange(
                        "(g p) c -> p g c", p=128
                    ),
                    in_=st[:].rearrange("p (g c) -> p g c", c=4),
                )

            # phase 2: edge grids
            for t in range(NT):
                dt = dts[t]
                r0 = t * 128
                idxt = epool.tile([128, dt], I32, tag="idxt")
                nc.sync.dma_start(out=idxt[:], in_=idx_g[r0 : r0 + 128, 0:dt])
                uix = spool.tile([128, 1], I32, tag="uix")
                nc.sync.dma_start(out=uix[:], in_=u_idx[r0 : r0 + 128, :])
                ut = spool.tile([128, 4], F32, tag="ut")
                nc.gpsimd.indirect_dma_start(
                    out=ut[:],
                    out_offset=None,
                    in_=uv[:],
                    in_offset=bass.IndirectOffsetOnAxis(ap=uix[:], axis=0),
                )
                vals = vpool.tile([128, dt * 4], F32, tag="vals")
                for j in range(dt):
                    nc.gpsimd.indirect_dma_start(
                        out=vals[:, j * 4 : (j + 1) * 4],
                        out_offset=None,
                        in_=uv[:],
                        in_offset=bass.IndirectOffsetOnAxis(
                            ap=idxt[:, j : j + 1], axis=0
                        ),
                    )
                vt = epool.tile([128, dt], F32, tag="vt")
                nc.sync.dma_start(out=vt[:], in_=valid_g[r0 : r0 + 128, 0:dt])
                mt = epool.tile([128, dt], F32, tag="mt")
                nc.sync.dma_start(out=mt[:], in_=mask_g[r0 : r0 + 128, 0:dt])

                v3 = vals[:].rearrange("p (d c) -> p d c", c=4)
                o = epool.tile([128, dt], F32, tag="o")
                den = spool.tile([128, 2], F32, tag="den")
                rec = spool.tile([128, 2], F32, tag="rec")
                for c in range(2):
                    ec = epool.tile([128, dt], F32, tag=f"e{c}")
                    # e = exp(relu(v + (u+b)))
                    nc.scalar.activation(
                        out=ec[:],
                        in_=v3[:, :, 2 + c],
                        func=mybir.ActivationFunctionType.Relu,
                        bias=ut[:, c : c + 1],
                    )
                    nc.scalar.activation(
                        out=ec[:], in_=ec[:], func=mybir.ActivationFunctionType.Exp
                    )
                    nc.vector.tensor_mul(out=ec[:], in0=ec[:], in1=vt[:])
                    nc.vector.tensor_reduce(
                        out=den[:, c : c + 1],
                        in_=ec[:],
                        axis=mybir.AxisListType.X,
                        op=mybir.AluOpType.add,
                    )
                    nc.vector.reciprocal(
                        out=rec[:, c : c + 1], in_=den[:, c : c + 1]
                    )
                    if c == 0:
                        nc.vector.tensor_scalar_mul(
                            out=o[:], in0=ec[:], scalar1=rec[:, 0:1]
                        )
                    else:
                        ec2 = epool.tile([128, dt], F32, tag="ec2")
                        nc.vector.tensor_scalar_mul(
                            out=ec2[:], in0=ec[:], scalar1=rec[:, 1:2]
                        )
                        nc.vector.tensor_add(out=o[:], in0=o[:], in1=ec2[:])
                nc.vector.tensor_mul(out=o[:], in0=o[:], in1=mt[:])
                nc.sync.dma_start(out=out_g[r0 : r0 + 128, 0:dt], in_=o[:])

    _split_waits(nc)
    return nc


# ---------------------------------------------------------------- host side
def kernel(x, edge_index, actual_amount, W, b):
    x = np.asarray(x, np.float32)
    edge_index = np.asarray(edge_index)
    amt = np.asarray(actual_amount).ravel()
    W = np.asarray(W, np.float32)
    b = np.asarray(b, np.float32)
    row = edge_index[0].astype(np.int64)
    col = edge_index[1].astype(np.int64)

    # replicated x, transposed+padded for PE (features on partitions)
    x_t = np.zeros((D, NROWS_TBL), np.float32)
    x_t[:, :N] = x.T
    # table columns: u0+b0, u1+b1, v0, v1
    wcat = np.stack([W[0, :D], W[1, :D], W[0, D:], W[1, D:]], axis=1).astype(
        np.float32
    )
    btile = np.tile(
        np.array([b[0], b[1], 0.0, 0.0], np.float32)[None, :], (128, 1)
    )

    per_core = []
    dts_all = np.zeros((NC, NT), np.int64)
    for c in range(NC):
        sel = np.nonzero((row >= c * RPC) & (row < (c + 1) * RPC))[0]
        r_loc = row[sel] - c * RPC
        deg = np.bincount(r_loc, minlength=RPC)
        perm = np.argsort(-deg, kind="stable")        # grid row -> local node
        inv = np.empty(RPC, np.int64)
        inv[perm] = np.arange(RPC)
        prow = inv[r_loc]                             # grid row per edge
        order = np.argsort(prow, kind="stable")
        sel_o = sel[order]
        prow_o = prow[order]
        counts = np.bincount(prow_o, minlength=RPC)
        offs = np.concatenate([[0], np.cumsum(counts)[:-1]])
        slot = np.arange(len(sel_o)) - offs[prow_o]
        deg_sorted = deg[perm]
        for t in range(NT):
            lo = t * 128
            dts_all[c, t] = deg_sorted[lo] if lo < RPC else 0
        per_core.append((sel_o, prow_o, slot, perm, deg_sorted))

    dts = tuple(int(max(1, d)) for d in dts_all.max(axis=0))
    W_slots = max(dts)

    key = dts
    if key not in _CACHE:
        _CACHE[key] = _build_nc(dts)
    nc = _CACHE[key]

    in_maps = []
    for c in range(NC):
        sel_o, prow_o, slot, perm, deg_sorted = per_core[c]
        idx_g = np.zeros((RP, W_slots), np.int32)
        valid_g = np.zeros((RP, W_slots), np.float32)
        mask_g = np.zeros((RP, W_slots), np.float32)
        idx_g[prow_o, slot] = col[sel_o].astype(np.int32)
        valid_g[prow_o, slot] = 1.0
        mask_g[prow_o, slot] = (amt[sel_o] != 0).astype(np.float32)
        u_idx = np.zeros((RP, 1), np.int32)
        u_idx[:RPC, 0] = (c * RPC + perm).astype(np.int32)
        in_maps.append(
            {
                "x_t": x_t,
                "wcat": wcat,
                "btile": btile,
                "idx_g": idx_g,
                "u_idx": u_idx,
                "valid_g": valid_g,
                "mask_g": mask_g,
            }
        )

    res = run_bass_kernel_spmd(nc, in_maps, list(range(NC)))

    out = np.zeros(E, np.float32)
    for c in range(NC):
        sel_o, prow_o, slot, _, _ = per_core[c]
        grid = np.asarray(res.results[c]["out_g"])
        out[sel_o] = grid[prow_o, slot]
    return out
